# revision 26
# baseline (speedup 1.0000x reference)
"""MultiOutSizeLinear (MoE-style routed linear) for Trainium2, 8 NeuronCores.

Each token selects one of 4 experts by its ``out_feat_size`` value
(128/256/512/1024). Expert k is a dense [out_k, 1024] linear + bias whose
output lands in the first out_k columns of the 1024-wide output row; the
reference leaves bias[k, out_k:] in the remaining columns (zero for the
shipped setup_inputs, which pre-zeroes the bias tail).

Strategy
  host:   route tokens to experts; balance each expert's tokens evenly
          across the 8 cores (capacities are shared so one SPMD program
          serves all cores); gather + transpose each core's tokens into
          x^T [1024, TPAD] laid out as expert segments [e3 | e2 | e1 | e0].
  device: keep W^T [1024, 1920] (all experts, concatenated out-columns) and
          a 128-row broadcast bias resident in SBUF. Stream 512-token
          chunks of x^T over the ACT HWDGE ring. All tensors that feed the
          PE are declared float32r (raw fp32 bits; the PE's full-rate fp32
          mode, ~1.3e-4 relative error vs fp32). Experts 1-3 run
          token-stationary: psum[128 tok, out_k] += xT_tile.T @ wT_tile,
          8 accumulating K-tiles per <=512-wide column chunk. Expert 0
          (out=128, too narrow for full-rate f32r) runs weight-stationary:
          psum[128 out, 512 tok] = out0^T chunks. Bias is added on VectorE
          during PSUM eviction (expert 0's bias is added on the host).
          Compact per-expert outputs go back over the SP HWDGE ring.
  host:   scatter rows back through the routing permutation.
"""

import sys
import numpy as np

sys.path.insert(0, "/opt/trn_rl_repo")

OUT_SIZES = (128, 256, 512, 1024)
N_EXPERTS = len(OUT_SIZES)
IN_FEAT = 1024
N_CORES = 8
K_TILES = IN_FEAT // 128
CHUNK = 512  # tokens per x^T DMA (CHUNK=1024 measured slower: 231us vs 212)
WOFF = tuple(int(np.cumsum((0,) + OUT_SIZES)[k]) for k in range(N_EXPERTS))
W_COLS = sum(OUT_SIZES)

_nc_cache: dict = {}

# x^T HBM layout: True = host-staged [128, K_TILES, CHUNK] linear per-
# partition runs; False = [IN_FEAT, CHUNK] slab with DMA-side rearrange.
# Measured: linear wins in isolated DMA benches but loses in-kernel (large
# packets serialize the two HWDGE rings); rearrange overlaps better.
X_LINEAR = False


def _build(caps, repeat=1, loop=None, xbufs=6, obufs=4):
    """Compile the SPMD program for shared per-expert capacities ``caps``.

    caps[0] % 512 == 0, caps[1]+caps[2]+caps[3] % 512 == 0, each % 128 == 0.
    ``repeat``/``loop`` re-run the compute body (same I/O) for timing.
    """
    import concourse.bacc as bacc
    import concourse.mybir as mybir
    import concourse.tile as tile

    f32 = mybir.dt.float32
    f16 = mybir.dt.float16
    tpad = sum(caps)
    assert tpad % CHUNK == 0 and caps[0] % 512 == 0
    assert (caps[1] + caps[2] + caps[3]) % 512 == 0

    nc = bacc.Bacc(None, target_bir_lowering=False, debug=False)
    # chunk-blocked x^T in fp16, host-staged in the exact SBUF layout
    # [128 part, K_TILES, CHUNK]: each chunk DMA is a pure linear copy with
    # one contiguous K_TILES*CHUNK*2B run per partition (fp16 halves HBM
    # traffic vs f32r; PE rate is identical at 1 cycle/row and accumulation
    # stays fp32 in PSUM)
    if X_LINEAR:
        xt = nc.dram_tensor("xt", [tpad // CHUNK, 128, K_TILES, CHUNK], f16,
                            kind="ExternalInput")
    else:
        xt = nc.dram_tensor("xt", [tpad // CHUNK, IN_FEAT, CHUNK], f16,
                            kind="ExternalInput")
    wt = nc.dram_tensor("wt", [IN_FEAT, W_COLS], f16, kind="ExternalInput")
    bb = nc.dram_tensor("bb", [128, W_COLS], f32, kind="ExternalInput")
    outs = {}
    for k in (1, 2, 3):
        if caps[k]:
            outs[k] = nc.dram_tensor(f"out{k}", [caps[k], OUT_SIZES[k]], f16,
                                     kind="ExternalOutput")
    if caps[0]:
        outs[0] = nc.dram_tensor("out0t", [128, caps[0]], f16,
                                 kind="ExternalOutput")

    seg_order = [k for k in (3, 2, 1, 0) if caps[k] > 0]
    seg_start = {}
    t0 = 0
    for k in seg_order:
        seg_start[k] = t0
        t0 += caps[k]

    def expert_of(tok):
        for k in seg_order:
            if tok < seg_start[k] + caps[k]:
                return k
        raise AssertionError


    with tile.TileContext(nc) as tc:
        with (
            tc.tile_pool(name="const", bufs=1) as const,
            tc.tile_pool(name="xp", bufs=xbufs) as xp,
            tc.tile_pool(name="op", bufs=obufs) as op,
            tc.tile_pool(name="ps", bufs=3, space="PSUM") as psp,
            tc.tile_pool(name="ps0", bufs=2, space="PSUM") as psp0,
        ):
            wt_sb = const.tile([128, K_TILES, W_COLS], f16)
            nc.sync.dma_start(wt_sb[:], wt.rearrange("(kk p) n -> p kk n", p=128))
            bb_sb = const.tile([128, W_COLS], f32)
            nc.sync.dma_start(bb_sb[:], bb[:])

            def body():
                for c0 in range(0, tpad, CHUNK):
                    x_sb = xp.tile([128, K_TILES, CHUNK], f16, tag="x")
                    if X_LINEAR:
                        nc.scalar.dma_start(x_sb[:], xt[c0 // CHUNK])
                    else:
                        nc.scalar.dma_start(
                            x_sb[:],
                            xt[c0 // CHUNK].rearrange("(kk p) t -> p kk t",
                                                      p=128))
                    g0 = 0
                    while g0 < CHUNK:
                        tok = c0 + g0
                        k = expert_of(tok)
                        if k == 0:
                            # weight-stationary: psum = out0^T [128 out, 512 tok]
                            ps = psp0.tile([128, 512], f32, tag="ps0")
                            for kk in range(K_TILES):
                                nc.tensor.matmul(
                                    ps[:],
                                    wt_sb[:, kk, WOFF[0]:WOFF[0] + 128],
                                    x_sb[:, kk, g0:g0 + 512],
                                    start=(kk == 0), stop=(kk == K_TILES - 1))
                            o_sb = op.tile([128, 512], f16, tag="o0")
                            nc.vector.tensor_copy(o_sb[:], ps[:])
                            row = tok - seg_start[0]
                            nc.sync.dma_start(outs[0][:, row:row + 512], o_sb[:])
                            g0 += 512
                            continue
                        ok = OUT_SIZES[k]
                        # run of same-expert 128-token groups in this chunk:
                        # stage their outputs side by side, flush as ONE DMA
                        # (56 separate per-group output DMAs serialize on the
                        # SP HWDGE ring's ~2.6us fixed cost)
                        seg_end = seg_start[k] + caps[k]
                        rl = min(seg_end - tok, CHUNK - g0) // 128
                        o_sb = op.tile([128, 4, 1024], f16, tag="o")
                        for g in range(rl):
                            gt = g0 + g * 128
                            ps = psp.tile([128, 1024], f32, tag="ps")
                            for j0 in range(0, ok, 512):
                                jn = min(512, ok - j0)
                                for kk in range(K_TILES):
                                    nc.tensor.matmul(
                                        ps[:, j0:j0 + jn],
                                        x_sb[:, kk, gt:gt + 128],
                                        wt_sb[:, kk,
                                              WOFF[k] + j0:WOFF[k] + j0 + jn],
                                        start=(kk == 0),
                                        stop=(kk == K_TILES - 1))
                            nc.vector.tensor_add(
                                o_sb[:, g, :ok], ps[:, :ok],
                                bb_sb[:, WOFF[k]:WOFF[k] + ok])
                        row = tok - seg_start[k]
                        nc.sync.dma_start(
                            outs[k][row:row + rl * 128, :]
                            .rearrange("(g p) n -> p g n", p=128),
                            o_sb[:, :rl, :ok])
                        g0 += rl * 128

            if loop:
                with tc.For_i(0, loop, 1):
                    body()
            else:
                for _ in range(repeat):
                    body()
    nc.compile()
    return nc


def _get_nc(caps, repeat=1, loop=None):
    key = (tuple(caps), repeat, loop)
    if key not in _nc_cache:
        _nc_cache[key] = _build(caps, repeat=repeat, loop=loop)
    return _nc_cache[key]


def _route(out_feat_size):
    """Map out_feat_size values -> expert index (-1 = matches no expert)."""
    ofs = np.asarray(out_feat_size).astype(np.int64).reshape(-1)
    branch = np.full(ofs.shape, -1, dtype=np.int64)
    for k, s in enumerate(OUT_SIZES):
        branch[ofs == s] = k
    return branch


def _plan(branch):
    """Balanced routing plan: per-expert global index lists split evenly
    across cores, shared capacities, and segment layout [3,2,1,0]."""
    idx_all = {k: np.nonzero(branch == k)[0] for k in range(N_EXPERTS)}
    per_core = [int(-(-len(idx_all[k]) // N_CORES)) for k in range(N_EXPERTS)]
    caps = [int(-(-per_core[k] // 128) * 128) for k in range(N_EXPERTS)]
    # alignment: caps0 % 512, (caps1+2+3) % 512
    if caps[0] % 512:
        caps[0] += 512 - caps[0] % 512
    rem = (caps[1] + caps[2] + caps[3]) % 512
    if rem:
        for k in (1, 2, 3):  # pad the cheapest non-empty of e1..e3
            if caps[k]:
                caps[k] += 512 - rem
                break
        else:
            caps[0] += (512 - rem) if caps[0] else 0
    # chunk alignment: pad e0 (cheapest per token) to make tpad % CHUNK
    rem2 = sum(caps) % CHUNK
    if rem2:
        pad = CHUNK - rem2
        if caps[0] % 512 == 0 and pad % 512 == 0:
            caps[0] += pad
        else:
            for k in (1, 2, 3):
                if caps[k]:
                    caps[k] += pad
                    break
    return idx_all, tuple(caps)


def _prepare(x, weight, bias, out_feat_size):
    """Host-side routing + input staging. Returns None if no tokens match."""
    x = np.asarray(x, dtype=np.float32)
    weight = np.asarray(weight, dtype=np.float32)
    bias = np.asarray(bias, dtype=np.float32)
    B, T, D = x.shape
    assert D == IN_FEAT
    n_tok = B * T

    branch = _route(out_feat_size)
    idx_all, caps = _plan(branch)
    if sum(caps) == 0:
        return None

    # host-side weight/bias layout (fp16 device tensors)
    wt = np.empty((IN_FEAT, W_COLS), dtype=np.float16)
    bb = np.empty((W_COLS,), dtype=np.float32)
    for k, ok in enumerate(OUT_SIZES):
        wt[:, WOFF[k]:WOFF[k] + ok] = weight[k, :ok, :].T
        bb[WOFF[k]:WOFF[k] + ok] = bias[k, :ok]
    bb128 = np.ascontiguousarray(np.broadcast_to(bb, (128, W_COLS)))

    x2 = x.reshape(n_tok, IN_FEAT).astype(np.float16)
    tpad = sum(caps)
    seg_off = {}
    t0 = 0
    for k in (3, 2, 1, 0):
        if caps[k]:
            seg_off[k] = t0
            t0 += caps[k]

    in_maps = []
    core_slices = []  # per core: {expert: global idx array}
    for c in range(N_CORES):
        perm = np.zeros(tpad, dtype=np.int64)
        slices = {}
        for k, off in seg_off.items():
            idx = idx_all[k]
            m = int(-(-len(idx) // N_CORES))
            part = idx[c * m:(c + 1) * m]
            slices[k] = part
            if len(part):
                perm[off:off + len(part)] = part
                perm[off + len(part):off + caps[k]] = part[0]
        if X_LINEAR:
            xtb = np.empty((tpad // CHUNK, 128, K_TILES, CHUNK),
                           dtype=np.float16)
            for ci in range(tpad // CHUNK):
                blk = x2[perm[ci * CHUNK:(ci + 1) * CHUNK]].T
                np.copyto(xtb[ci], blk.reshape(K_TILES, 128, CHUNK)
                          .transpose(1, 0, 2))
        else:
            xtb = np.empty((tpad // CHUNK, IN_FEAT, CHUNK), dtype=np.float16)
            for ci in range(tpad // CHUNK):
                np.copyto(xtb[ci], x2[perm[ci * CHUNK:(ci + 1) * CHUNK]].T)
        in_maps.append({"xt": xtb, "wt": wt, "bb": bb128})
        core_slices.append(slices)

    return dict(caps=caps, in_maps=in_maps, core_slices=core_slices,
                bias=bias, n_tok=n_tok, shape=(B, T))


def _scatter(prep, res):
    """Scatter per-core compact expert outputs back to full [B,T,1024]."""
    bias = prep["bias"]
    B, T = prep["shape"]
    out = np.zeros((prep["n_tok"], IN_FEAT), dtype=np.float32)
    for c in range(N_CORES):
        for k, part in prep["core_slices"][c].items():
            n = len(part)
            if n == 0:
                continue
            ok = OUT_SIZES[k]
            if k == 0:
                out[part, :ok] = res[c]["out0t"][:, :n].T.astype(np.float32) \
                    + bias[0, :ok]
            else:
                out[part, :ok] = res[c][f"out{k}"][:n]
            if ok < IN_FEAT:
                # reference semantics: bias tail beyond out_k (zero for the
                # shipped inputs, which pre-zero the bias)
                out[part, ok:] = bias[k, ok:]
    return out.reshape(B, T, IN_FEAT)


def kernel(x, weight, bias, out_feat_size):
    from concourse.bass_utils import run_bass_kernel_spmd

    prep = _prepare(x, weight, bias, out_feat_size)
    if prep is None:
        B, T, _ = np.asarray(x).shape
        return np.zeros((B, T, IN_FEAT), dtype=np.float32)

    global _LAST_CAPS, _LAST_IN_MAPS
    _LAST_CAPS, _LAST_IN_MAPS = prep["caps"], prep["in_maps"]

    nc = _get_nc(prep["caps"])
    res = run_bass_kernel_spmd(nc, prep["in_maps"],
                               list(range(N_CORES))).results
    return _scatter(prep, res)



# revision 28
# speedup vs baseline: 1.0585x; 1.0585x over previous
"""MultiOutSizeLinear (MoE-style routed linear) for Trainium2, 8 NeuronCores.

Each token selects one of 4 experts by its ``out_feat_size`` value
(128/256/512/1024). Expert k is a dense [out_k, 1024] linear + bias whose
output lands in the first out_k columns of the 1024-wide output row; the
reference leaves bias[k, out_k:] in the remaining columns (zero for the
shipped setup_inputs, which pre-zeroes the bias tail).

Strategy
  host:   route tokens to experts; balance each expert's tokens evenly
          across the 8 cores (capacities are shared so one SPMD program
          serves all cores); gather + transpose each core's tokens into
          x^T [1024, TPAD] laid out as expert segments [e3 | e2 | e1 | e0],
          cast to fp16 (PE runs fp16 at full rate like bf16, accumulation
          stays fp32 in PSUM; end-to-end rel err ~5e-4, halves HBM traffic
          vs f32r).
  device: keep W^T [1024, 1920] fp16 (all experts, concatenated
          out-columns) and a 128-row broadcast fp32 bias resident in SBUF.
          Stream 512-token chunks of x^T over the ACT HWDGE ring. Experts
          1-3 run token-stationary: psum[128 tok, out_k] += xT_tile.T @
          wT_tile, 8 accumulating K-tiles per <=512-wide column chunk.
          Expert 0 (out=128) runs weight-stationary: psum[128 out, 512 tok]
          = out0^T chunks. Bias is added on VectorE during PSUM eviction
          with fp16 output cast (expert 0's bias is added on the host).
          Compact per-expert fp16 outputs go back over the SP HWDGE ring.
  host:   scatter rows back through the routing permutation, upcast fp32.

Measured (8-core SPMD, loop-timing): f32r original 247.6us; this fp16
version 212.1us. Variants tried and measured SLOWER in-kernel despite
winning isolated microbenches (all 227-250us): host-linear x layout,
single-bank [128,512] psum tiles w/ 8-deep rotation, CHUNK=1024,
chunk-merged output DMAs, deeper x/out buffer pools.
"""

import sys
import numpy as np

sys.path.insert(0, "/opt/trn_rl_repo")

OUT_SIZES = (128, 256, 512, 1024)
N_EXPERTS = len(OUT_SIZES)
IN_FEAT = 1024
N_CORES = 8
K_TILES = IN_FEAT // 128
CHUNK = 512  # tokens per x^T DMA (CHUNK=1024 measured slower: 231us vs 212)
WOFF = tuple(int(np.cumsum((0,) + OUT_SIZES)[k]) for k in range(N_EXPERTS))
W_COLS = sum(OUT_SIZES)

_nc_cache: dict = {}

# x^T HBM layout: True = host-staged [128, K_TILES, CHUNK] linear per-
# partition runs; False = [IN_FEAT, CHUNK] slab with DMA-side rearrange.
# Measured: linear wins in isolated DMA benches but loses in-kernel (large
# packets serialize the two HWDGE rings); rearrange overlaps better.
X_LINEAR = False


def _build(caps, repeat=1, loop=None, xbufs=6, obufs=4):
    """Compile the SPMD program for shared per-expert capacities ``caps``.

    caps[0] % 512 == 0, caps[1]+caps[2]+caps[3] % 512 == 0, each % 128 == 0.
    ``repeat``/``loop`` re-run the compute body (same I/O) for timing.
    """
    import concourse.bacc as bacc
    import concourse.mybir as mybir
    import concourse.tile as tile

    f32 = mybir.dt.float32
    f16 = mybir.dt.float16
    tpad = sum(caps)
    assert tpad % CHUNK == 0 and caps[0] % 512 == 0
    assert (caps[1] + caps[2] + caps[3]) % 512 == 0

    nc = bacc.Bacc(None, target_bir_lowering=False, debug=False)
    # chunk-blocked x^T in fp16, host-staged in the exact SBUF layout
    # [128 part, K_TILES, CHUNK]: each chunk DMA is a pure linear copy with
    # one contiguous K_TILES*CHUNK*2B run per partition (fp16 halves HBM
    # traffic vs f32r; PE rate is identical at 1 cycle/row and accumulation
    # stays fp32 in PSUM)
    if X_LINEAR:
        xt = nc.dram_tensor("xt", [tpad // CHUNK, 128, K_TILES, CHUNK], f16,
                            kind="ExternalInput")
    else:
        xt = nc.dram_tensor("xt", [tpad // CHUNK, IN_FEAT, CHUNK], f16,
                            kind="ExternalInput")
    wt = nc.dram_tensor("wt", [IN_FEAT, W_COLS], f16, kind="ExternalInput")
    bb = nc.dram_tensor("bb", [128, W_COLS], f32, kind="ExternalInput")
    outs = {}
    for k in (1, 2, 3):
        if caps[k]:
            outs[k] = nc.dram_tensor(f"out{k}", [caps[k], OUT_SIZES[k]], f16,
                                     kind="ExternalOutput")
    if caps[0]:
        outs[0] = nc.dram_tensor("out0t", [128, caps[0]], f16,
                                 kind="ExternalOutput")

    seg_order = [k for k in (3, 2, 1, 0) if caps[k] > 0]
    seg_start = {}
    t0 = 0
    for k in seg_order:
        seg_start[k] = t0
        t0 += caps[k]

    def expert_of(tok):
        for k in seg_order:
            if tok < seg_start[k] + caps[k]:
                return k
        raise AssertionError


    with tile.TileContext(nc) as tc:
        with (
            tc.tile_pool(name="const", bufs=1) as const,
            tc.tile_pool(name="xp", bufs=xbufs) as xp,
            tc.tile_pool(name="op", bufs=obufs) as op,
            tc.tile_pool(name="ps", bufs=3, space="PSUM") as psp,
            tc.tile_pool(name="ps0", bufs=2, space="PSUM") as psp0,
        ):
            wt_sb = const.tile([128, K_TILES, W_COLS], f16)
            nc.sync.dma_start(wt_sb[:], wt.rearrange("(kk p) n -> p kk n", p=128))
            bb_sb = const.tile([128, W_COLS], f32)
            nc.sync.dma_start(bb_sb[:], bb[:])

            def body():
                for c0 in range(0, tpad, CHUNK):
                    x_sb = xp.tile([128, K_TILES, CHUNK], f16, tag="x")
                    if X_LINEAR:
                        nc.scalar.dma_start(x_sb[:], xt[c0 // CHUNK])
                    else:
                        nc.scalar.dma_start(
                            x_sb[:],
                            xt[c0 // CHUNK].rearrange("(kk p) t -> p kk t",
                                                      p=128))
                    g0 = 0
                    while g0 < CHUNK:
                        tok = c0 + g0
                        k = expert_of(tok)
                        if k == 0:
                            # weight-stationary: psum = out0^T [128 out, 512 tok]
                            ps = psp0.tile([128, 512], f32, tag="ps0")
                            for kk in range(K_TILES):
                                nc.tensor.matmul(
                                    ps[:],
                                    wt_sb[:, kk, WOFF[0]:WOFF[0] + 128],
                                    x_sb[:, kk, g0:g0 + 512],
                                    start=(kk == 0), stop=(kk == K_TILES - 1))
                            o_sb = op.tile([128, 512], f16, tag="o0")
                            nc.vector.tensor_copy(o_sb[:], ps[:])
                            row = tok - seg_start[0]
                            nc.sync.dma_start(outs[0][:, row:row + 512], o_sb[:])
                            g0 += 512
                            continue
                        ok = OUT_SIZES[k]
                        ps = psp.tile([128, 1024], f32, tag="ps")
                        for j0 in range(0, ok, 512):
                            jn = min(512, ok - j0)
                            for kk in range(K_TILES):
                                nc.tensor.matmul(
                                    ps[:, j0:j0 + jn],
                                    x_sb[:, kk, g0:g0 + 128],
                                    wt_sb[:, kk, WOFF[k] + j0:WOFF[k] + j0 + jn],
                                    start=(kk == 0), stop=(kk == K_TILES - 1))
                        o_sb = op.tile([128, 1024], f16, tag="o")
                        nc.vector.tensor_add(o_sb[:, :ok], ps[:, :ok],
                                             bb_sb[:, WOFF[k]:WOFF[k] + ok])
                        row = tok - seg_start[k]
                        nc.sync.dma_start(outs[k][row:row + 128, :], o_sb[:, :ok])
                        g0 += 128

            if loop:
                with tc.For_i(0, loop, 1):
                    body()
            else:
                for _ in range(repeat):
                    body()
    nc.compile()
    return nc


def _get_nc(caps, repeat=1, loop=None):
    key = (tuple(caps), repeat, loop)
    if key not in _nc_cache:
        _nc_cache[key] = _build(caps, repeat=repeat, loop=loop)
    return _nc_cache[key]


def _route(out_feat_size):
    """Map out_feat_size values -> expert index (-1 = matches no expert)."""
    ofs = np.asarray(out_feat_size).astype(np.int64).reshape(-1)
    branch = np.full(ofs.shape, -1, dtype=np.int64)
    for k, s in enumerate(OUT_SIZES):
        branch[ofs == s] = k
    return branch


def _plan(branch):
    """Balanced routing plan: per-expert global index lists split evenly
    across cores, shared capacities, and segment layout [3,2,1,0]."""
    idx_all = {k: np.nonzero(branch == k)[0] for k in range(N_EXPERTS)}
    per_core = [int(-(-len(idx_all[k]) // N_CORES)) for k in range(N_EXPERTS)]
    caps = [int(-(-per_core[k] // 128) * 128) for k in range(N_EXPERTS)]
    # alignment: caps0 % 512, (caps1+2+3) % 512
    if caps[0] % 512:
        caps[0] += 512 - caps[0] % 512
    rem = (caps[1] + caps[2] + caps[3]) % 512
    if rem:
        for k in (1, 2, 3):  # pad the cheapest non-empty of e1..e3
            if caps[k]:
                caps[k] += 512 - rem
                break
        else:
            caps[0] += (512 - rem) if caps[0] else 0
    # chunk alignment: pad e0 (cheapest per token) to make tpad % CHUNK
    rem2 = sum(caps) % CHUNK
    if rem2:
        pad = CHUNK - rem2
        if caps[0] % 512 == 0 and pad % 512 == 0:
            caps[0] += pad
        else:
            for k in (1, 2, 3):
                if caps[k]:
                    caps[k] += pad
                    break
    return idx_all, tuple(caps)


def _prepare(x, weight, bias, out_feat_size):
    """Host-side routing + input staging. Returns None if no tokens match."""
    x = np.asarray(x, dtype=np.float32)
    weight = np.asarray(weight, dtype=np.float32)
    bias = np.asarray(bias, dtype=np.float32)
    B, T, D = x.shape
    assert D == IN_FEAT
    n_tok = B * T

    branch = _route(out_feat_size)
    idx_all, caps = _plan(branch)
    if sum(caps) == 0:
        return None

    # host-side weight/bias layout (fp16 device tensors)
    wt = np.empty((IN_FEAT, W_COLS), dtype=np.float16)
    bb = np.empty((W_COLS,), dtype=np.float32)
    for k, ok in enumerate(OUT_SIZES):
        wt[:, WOFF[k]:WOFF[k] + ok] = weight[k, :ok, :].T
        bb[WOFF[k]:WOFF[k] + ok] = bias[k, :ok]
    bb128 = np.ascontiguousarray(np.broadcast_to(bb, (128, W_COLS)))

    x2 = x.reshape(n_tok, IN_FEAT).astype(np.float16)
    tpad = sum(caps)
    seg_off = {}
    t0 = 0
    for k in (3, 2, 1, 0):
        if caps[k]:
            seg_off[k] = t0
            t0 += caps[k]

    in_maps = []
    core_slices = []  # per core: {expert: global idx array}
    for c in range(N_CORES):
        perm = np.zeros(tpad, dtype=np.int64)
        slices = {}
        for k, off in seg_off.items():
            idx = idx_all[k]
            m = int(-(-len(idx) // N_CORES))
            part = idx[c * m:(c + 1) * m]
            slices[k] = part
            if len(part):
                perm[off:off + len(part)] = part
                perm[off + len(part):off + caps[k]] = part[0]
        if X_LINEAR:
            xtb = np.empty((tpad // CHUNK, 128, K_TILES, CHUNK),
                           dtype=np.float16)
            for ci in range(tpad // CHUNK):
                blk = x2[perm[ci * CHUNK:(ci + 1) * CHUNK]].T
                np.copyto(xtb[ci], blk.reshape(K_TILES, 128, CHUNK)
                          .transpose(1, 0, 2))
        else:
            xtb = np.empty((tpad // CHUNK, IN_FEAT, CHUNK), dtype=np.float16)
            for ci in range(tpad // CHUNK):
                np.copyto(xtb[ci], x2[perm[ci * CHUNK:(ci + 1) * CHUNK]].T)
        in_maps.append({"xt": xtb, "wt": wt, "bb": bb128})
        core_slices.append(slices)

    return dict(caps=caps, in_maps=in_maps, core_slices=core_slices,
                bias=bias, n_tok=n_tok, shape=(B, T))


def _scatter(prep, res):
    """Scatter per-core compact expert outputs back to full [B,T,1024]."""
    bias = prep["bias"]
    B, T = prep["shape"]
    out = np.zeros((prep["n_tok"], IN_FEAT), dtype=np.float32)
    for c in range(N_CORES):
        for k, part in prep["core_slices"][c].items():
            n = len(part)
            if n == 0:
                continue
            ok = OUT_SIZES[k]
            if k == 0:
                out[part, :ok] = res[c]["out0t"][:, :n].T.astype(np.float32) \
                    + bias[0, :ok]
            else:
                out[part, :ok] = res[c][f"out{k}"][:n]
            if ok < IN_FEAT:
                # reference semantics: bias tail beyond out_k (zero for the
                # shipped inputs, which pre-zero the bias)
                out[part, ok:] = bias[k, ok:]
    return out.reshape(B, T, IN_FEAT)


def kernel(x, weight, bias, out_feat_size):
    from concourse.bass_utils import run_bass_kernel_spmd

    prep = _prepare(x, weight, bias, out_feat_size)
    if prep is None:
        B, T, _ = np.asarray(x).shape
        return np.zeros((B, T, IN_FEAT), dtype=np.float32)

    global _LAST_CAPS, _LAST_IN_MAPS
    _LAST_CAPS, _LAST_IN_MAPS = prep["caps"], prep["in_maps"]

    nc = _get_nc(prep["caps"])
    res = run_bass_kernel_spmd(nc, prep["in_maps"],
                               list(range(N_CORES))).results
    return _scatter(prep, res)



# revision 35
# speedup vs baseline: 1.0826x; 1.0228x over previous
"""MultiOutSizeLinear (MoE-style routed linear) for Trainium2, 8 NeuronCores.

Each token selects one of 4 experts by its ``out_feat_size`` value
(128/256/512/1024). Expert k is a dense [out_k, 1024] linear + bias whose
output lands in the first out_k columns of the 1024-wide output row; the
reference leaves bias[k, out_k:] in the remaining columns (zero for the
shipped setup_inputs, which pre-zeroes the bias tail).

Strategy
  host:   route tokens to experts; balance each expert's tokens evenly
          across the 8 cores (capacities are shared so one SPMD program
          serves all cores); gather + transpose each core's tokens into
          x^T [1024, TPAD] laid out as expert segments [e3 | e2 | e1 | e0],
          cast to fp16 (PE runs fp16 at full rate like bf16, accumulation
          stays fp32 in PSUM; end-to-end rel err ~5e-4, halves HBM traffic
          vs f32r).
  device: keep W^T [1024, 1920] fp16 (all experts, concatenated
          out-columns) and a 128-row broadcast fp32 bias resident in SBUF.
          Stream 512-token chunks of x^T over the ACT HWDGE ring. Experts
          1-3 run token-stationary: psum[128 tok, out_k] += xT_tile.T @
          wT_tile, 8 accumulating K-tiles per <=512-wide column chunk.
          Expert 0 (out=128) runs weight-stationary: psum[128 out, 512 tok]
          = out0^T chunks. Bias is added on VectorE during PSUM eviction
          with fp16 output cast (expert 0's bias is added on the host).
          Compact per-expert fp16 outputs go back over the SP HWDGE ring.
  host:   scatter rows back through the routing permutation, upcast fp32.

Measured (8-core SPMD, loop-timing; note +-10% cross-process variance):
f32r original 247.6us; this fp16 version 212-236us (best 211.8).
Paired in-process A/B confirmed this config beats single-bank [128,512]
psum x8 rotation (psum1bank=True flag) by ~15-25us median, and deeper
x/out pools (xbufs=10/obufs=8) are a wash. Also tried and slower in
single runs: host-linear x layout (X_LINEAR=True), CHUNK=1024,
chunk-merged output DMAs. Isolated microbenches (pure-MM rate, DMA-only
patterns) did NOT predict in-kernel ranking; trust paired kernel runs.
"""

import sys
import numpy as np

sys.path.insert(0, "/opt/trn_rl_repo")

OUT_SIZES = (128, 256, 512, 1024)
N_EXPERTS = len(OUT_SIZES)
IN_FEAT = 1024
N_CORES = 8
K_TILES = IN_FEAT // 128
CHUNK = 512  # tokens per x^T DMA (CHUNK=1024 measured slower: 231us vs 212)
WOFF = tuple(int(np.cumsum((0,) + OUT_SIZES)[k]) for k in range(N_EXPERTS))
W_COLS = sum(OUT_SIZES)

_nc_cache: dict = {}

# x^T HBM layout: True = host-staged [128, K_TILES, CHUNK] linear per-
# partition runs; False = [IN_FEAT, CHUNK] slab with DMA-side rearrange.
# Measured: linear wins in isolated DMA benches but loses in-kernel (large
# packets serialize the two HWDGE rings); rearrange overlaps better.
X_LINEAR = False


def _build(caps, repeat=1, loop=None, xbufs=6, obufs=4, psum1bank=False):
    """Compile the SPMD program for shared per-expert capacities ``caps``.

    caps[0] % 512 == 0, caps[1]+caps[2]+caps[3] % 512 == 0, each % 128 == 0.
    ``repeat``/``loop`` re-run the compute body (same I/O) for timing.
    """
    import concourse.bacc as bacc
    import concourse.mybir as mybir
    import concourse.tile as tile

    f32 = mybir.dt.float32
    f16 = mybir.dt.float16
    tpad = sum(caps)
    assert tpad % CHUNK == 0 and caps[0] % 512 == 0
    assert (caps[1] + caps[2] + caps[3]) % 512 == 0

    nc = bacc.Bacc(None, target_bir_lowering=False, debug=False)
    # chunk-blocked x^T in fp16, host-staged in the exact SBUF layout
    # [128 part, K_TILES, CHUNK]: each chunk DMA is a pure linear copy with
    # one contiguous K_TILES*CHUNK*2B run per partition (fp16 halves HBM
    # traffic vs f32r; PE rate is identical at 1 cycle/row and accumulation
    # stays fp32 in PSUM)
    if X_LINEAR:
        xt = nc.dram_tensor("xt", [tpad // CHUNK, 128, K_TILES, CHUNK], f16,
                            kind="ExternalInput")
    else:
        xt = nc.dram_tensor("xt", [tpad // CHUNK, IN_FEAT, CHUNK], f16,
                            kind="ExternalInput")
    wt = nc.dram_tensor("wt", [IN_FEAT, W_COLS], f16, kind="ExternalInput")
    bb = nc.dram_tensor("bb", [128, W_COLS], f32, kind="ExternalInput")
    outs = {}
    for k in (1, 2, 3):
        if caps[k]:
            outs[k] = nc.dram_tensor(f"out{k}", [caps[k], OUT_SIZES[k]], f16,
                                     kind="ExternalOutput")
    if caps[0]:
        outs[0] = nc.dram_tensor("out0t", [128, caps[0]], f16,
                                 kind="ExternalOutput")

    seg_order = [k for k in (3, 2, 1, 0) if caps[k] > 0]
    seg_start = {}
    t0 = 0
    for k in seg_order:
        seg_start[k] = t0
        t0 += caps[k]

    def expert_of(tok):
        for k in seg_order:
            if tok < seg_start[k] + caps[k]:
                return k
        raise AssertionError


    from contextlib import ExitStack
    with tile.TileContext(nc) as tc, ExitStack() as stack:
        if True:
            const = stack.enter_context(tc.tile_pool(name="const", bufs=1))
            xp = stack.enter_context(tc.tile_pool(name="xp", bufs=xbufs))
            op = stack.enter_context(tc.tile_pool(name="op", bufs=obufs))
            psp = stack.enter_context(
                tc.tile_pool(name="ps", bufs=(8 if psum1bank else 3),
                             space="PSUM"))
            psp0 = psp if psum1bank else stack.enter_context(
                tc.tile_pool(name="ps0", bufs=2, space="PSUM"))
            wt_sb = const.tile([128, K_TILES, W_COLS], f16)
            nc.sync.dma_start(wt_sb[:], wt.rearrange("(kk p) n -> p kk n", p=128))
            bb_sb = const.tile([128, W_COLS], f32)
            nc.sync.dma_start(bb_sb[:], bb[:])

            def body():
                for c0 in range(0, tpad, CHUNK):
                    x_sb = xp.tile([128, K_TILES, CHUNK], f16, tag="x")
                    if X_LINEAR:
                        nc.scalar.dma_start(x_sb[:], xt[c0 // CHUNK])
                    else:
                        nc.scalar.dma_start(
                            x_sb[:],
                            xt[c0 // CHUNK].rearrange("(kk p) t -> p kk t",
                                                      p=128))
                    g0 = 0
                    while g0 < CHUNK:
                        tok = c0 + g0
                        k = expert_of(tok)
                        if k == 0:
                            # weight-stationary: psum = out0^T [128 out, 512 tok]
                            ps = psp0.tile([128, 512], f32,
                                           tag="ps" if psum1bank else "ps0")
                            for kk in range(K_TILES):
                                nc.tensor.matmul(
                                    ps[:],
                                    wt_sb[:, kk, WOFF[0]:WOFF[0] + 128],
                                    x_sb[:, kk, g0:g0 + 512],
                                    start=(kk == 0), stop=(kk == K_TILES - 1))
                            o_sb = op.tile([128, 512], f16, tag="o0")
                            nc.vector.tensor_copy(o_sb[:], ps[:])
                            row = tok - seg_start[0]
                            nc.sync.dma_start(outs[0][:, row:row + 512], o_sb[:])
                            g0 += 512
                            continue
                        ok = OUT_SIZES[k]
                        o_sb = op.tile([128, 1024], f16, tag="o")
                        if psum1bank:
                            # single-bank [128,512] psum tiles, 8-deep pool
                            pss = []
                            for _j in range(max(1, ok // 512)):
                                ps_j = psp.tile([128, 512], f32, tag="ps")
                                pss.append(ps_j)
                            for kk in range(K_TILES):
                                st, sp = kk == 0, kk == K_TILES - 1
                                for j, ps in enumerate(pss):
                                    jn = min(512, ok - j * 512)
                                    nc.tensor.matmul(
                                        ps[:, :jn],
                                        x_sb[:, kk, g0:g0 + 128],
                                        wt_sb[:, kk, WOFF[k] + j * 512:
                                              WOFF[k] + j * 512 + jn],
                                        start=st, stop=sp)
                            for j, ps in enumerate(pss):
                                jn = min(512, ok - j * 512)
                                nc.vector.tensor_add(
                                    o_sb[:, j * 512:j * 512 + jn],
                                    ps[:, :jn],
                                    bb_sb[:, WOFF[k] + j * 512:
                                          WOFF[k] + j * 512 + jn])
                        else:
                            ps = psp.tile([128, 1024], f32, tag="ps")
                            for j0 in range(0, ok, 512):
                                jn = min(512, ok - j0)
                                for kk in range(K_TILES):
                                    nc.tensor.matmul(
                                        ps[:, j0:j0 + jn],
                                        x_sb[:, kk, g0:g0 + 128],
                                        wt_sb[:, kk,
                                              WOFF[k] + j0:WOFF[k] + j0 + jn],
                                        start=(kk == 0),
                                        stop=(kk == K_TILES - 1))
                            nc.vector.tensor_add(o_sb[:, :ok], ps[:, :ok],
                                                 bb_sb[:, WOFF[k]:WOFF[k] + ok])
                        row = tok - seg_start[k]
                        nc.sync.dma_start(outs[k][row:row + 128, :], o_sb[:, :ok])
                        g0 += 128

            if loop:
                with tc.For_i(0, loop, 1):
                    body()
            else:
                for _ in range(repeat):
                    body()
    nc.compile()
    return nc


def _get_nc(caps, repeat=1, loop=None):
    key = (tuple(caps), repeat, loop)
    if key not in _nc_cache:
        _nc_cache[key] = _build(caps, repeat=repeat, loop=loop)
    return _nc_cache[key]


def _route(out_feat_size):
    """Map out_feat_size values -> expert index (-1 = matches no expert)."""
    ofs = np.asarray(out_feat_size).astype(np.int64).reshape(-1)
    branch = np.full(ofs.shape, -1, dtype=np.int64)
    for k, s in enumerate(OUT_SIZES):
        branch[ofs == s] = k
    return branch


def _plan(branch):
    """Balanced routing plan: per-expert global index lists split evenly
    across cores, shared capacities, and segment layout [3,2,1,0]."""
    idx_all = {k: np.nonzero(branch == k)[0] for k in range(N_EXPERTS)}
    per_core = [int(-(-len(idx_all[k]) // N_CORES)) for k in range(N_EXPERTS)]
    caps = [int(-(-per_core[k] // 128) * 128) for k in range(N_EXPERTS)]
    # alignment: caps0 % 512, (caps1+2+3) % 512
    if caps[0] % 512:
        caps[0] += 512 - caps[0] % 512
    rem = (caps[1] + caps[2] + caps[3]) % 512
    if rem:
        for k in (1, 2, 3):  # pad the cheapest non-empty of e1..e3
            if caps[k]:
                caps[k] += 512 - rem
                break
        else:
            caps[0] += (512 - rem) if caps[0] else 0
    # chunk alignment: pad e0 (cheapest per token) to make tpad % CHUNK
    rem2 = sum(caps) % CHUNK
    if rem2:
        pad = CHUNK - rem2
        if caps[0] % 512 == 0 and pad % 512 == 0:
            caps[0] += pad
        else:
            for k in (1, 2, 3):
                if caps[k]:
                    caps[k] += pad
                    break
    return idx_all, tuple(caps)


def _prepare(x, weight, bias, out_feat_size):
    """Host-side routing + input staging. Returns None if no tokens match."""
    x = np.asarray(x, dtype=np.float32)
    weight = np.asarray(weight, dtype=np.float32)
    bias = np.asarray(bias, dtype=np.float32)
    B, T, D = x.shape
    assert D == IN_FEAT
    n_tok = B * T

    branch = _route(out_feat_size)
    idx_all, caps = _plan(branch)
    if sum(caps) == 0:
        return None

    # host-side weight/bias layout (fp16 device tensors)
    wt = np.empty((IN_FEAT, W_COLS), dtype=np.float16)
    bb = np.empty((W_COLS,), dtype=np.float32)
    for k, ok in enumerate(OUT_SIZES):
        wt[:, WOFF[k]:WOFF[k] + ok] = weight[k, :ok, :].T
        bb[WOFF[k]:WOFF[k] + ok] = bias[k, :ok]
    bb128 = np.ascontiguousarray(np.broadcast_to(bb, (128, W_COLS)))

    x2 = x.reshape(n_tok, IN_FEAT).astype(np.float16)
    tpad = sum(caps)
    seg_off = {}
    t0 = 0
    for k in (3, 2, 1, 0):
        if caps[k]:
            seg_off[k] = t0
            t0 += caps[k]

    in_maps = []
    core_slices = []  # per core: {expert: global idx array}
    for c in range(N_CORES):
        perm = np.zeros(tpad, dtype=np.int64)
        slices = {}
        for k, off in seg_off.items():
            idx = idx_all[k]
            m = int(-(-len(idx) // N_CORES))
            part = idx[c * m:(c + 1) * m]
            slices[k] = part
            if len(part):
                perm[off:off + len(part)] = part
                perm[off + len(part):off + caps[k]] = part[0]
        if X_LINEAR:
            xtb = np.empty((tpad // CHUNK, 128, K_TILES, CHUNK),
                           dtype=np.float16)
            for ci in range(tpad // CHUNK):
                blk = x2[perm[ci * CHUNK:(ci + 1) * CHUNK]].T
                np.copyto(xtb[ci], blk.reshape(K_TILES, 128, CHUNK)
                          .transpose(1, 0, 2))
        else:
            xtb = np.empty((tpad // CHUNK, IN_FEAT, CHUNK), dtype=np.float16)
            for ci in range(tpad // CHUNK):
                np.copyto(xtb[ci], x2[perm[ci * CHUNK:(ci + 1) * CHUNK]].T)
        in_maps.append({"xt": xtb, "wt": wt, "bb": bb128})
        core_slices.append(slices)

    return dict(caps=caps, in_maps=in_maps, core_slices=core_slices,
                bias=bias, n_tok=n_tok, shape=(B, T))


def _scatter(prep, res):
    """Scatter per-core compact expert outputs back to full [B,T,1024]."""
    bias = prep["bias"]
    B, T = prep["shape"]
    out = np.zeros((prep["n_tok"], IN_FEAT), dtype=np.float32)
    for c in range(N_CORES):
        for k, part in prep["core_slices"][c].items():
            n = len(part)
            if n == 0:
                continue
            ok = OUT_SIZES[k]
            if k == 0:
                out[part, :ok] = res[c]["out0t"][:, :n].T.astype(np.float32) \
                    + bias[0, :ok]
            else:
                out[part, :ok] = res[c][f"out{k}"][:n]
            if ok < IN_FEAT:
                # reference semantics: bias tail beyond out_k (zero for the
                # shipped inputs, which pre-zero the bias)
                out[part, ok:] = bias[k, ok:]
    return out.reshape(B, T, IN_FEAT)


def kernel(x, weight, bias, out_feat_size):
    from concourse.bass_utils import run_bass_kernel_spmd

    prep = _prepare(x, weight, bias, out_feat_size)
    if prep is None:
        B, T, _ = np.asarray(x).shape
        return np.zeros((B, T, IN_FEAT), dtype=np.float32)

    global _LAST_CAPS, _LAST_IN_MAPS
    _LAST_CAPS, _LAST_IN_MAPS = prep["caps"], prep["in_maps"]

    nc = _get_nc(prep["caps"])
    res = run_bass_kernel_spmd(nc, prep["in_maps"],
                               list(range(N_CORES))).results
    return _scatter(prep, res)

